# revision 38
# baseline (speedup 1.0000x reference)
"""Multi-head attention (B=16, N=1024, C=768, H=12) on 8 TRN2 NeuronCores.

Strategy: pure data-parallel over batch (2 batches per core, no collectives).
All matmuls run in bf16 (1 PE cycle/row vs 4 for fp32; rel err ~6e-3).

Per-core pipeline, per batch b (layouts chosen so no transposes are needed):
  1. qkT  [1536, 1024]  = w_qkv[0:1536] @ x[b].T        (feature-major Q,K)
  2. vaug [1024, 12*65] = x[b] @ w_qkv[1536:].T         (token-major V, with
     a ones-column per head -> softmax denominators fall out of the PV matmul)
  3. per head h: S.T = kT.T @ qT (PE), P = exp(S.T * scale) (ACT, no
     max-subtraction needed: logits ~ N(0,1)), PV: outT[65, q] = vaug.T @ P
     accumulated over k tiles.  Row 64 of PV psum = softmax denominator.
     Normalize: reciprocal_approx_fast on the denom row, bf16 cast,
     broadcast across 64 partitions via a K=1 ones matmul, multiply.
  4. proj: out[tok, 768] = attn_outT.T @ w_proj.T + bias (bias folded into
     the matmul as an extra K=1 ones row).

PE/ACT overlap: attention is ACT(exp)-bound, so the projection matmul groups
of the previous batch and the QKV matmul groups of the next batch are
interleaved into the attention head loop via a pending-work queue.
"""

from collections import deque

import numpy as np
import ml_dtypes

B, N, C = 16, 1024, 768
H, HD = 12, 64
NCORES = 8
BL = B // NCORES  # batches per core
SCALE = HD ** -0.5

BF16 = ml_dtypes.bfloat16


def _build_graph():
    import concourse.mybir as mybir
    import concourse.tile as tile
    from concourse import bacc
    from concourse.bass import ds
    from contextlib import ExitStack

    bf = mybir.dt.bfloat16
    f32 = mybir.dt.float32
    Exp = mybir.ActivationFunctionType.Exp

    nc = bacc.Bacc(
        "TRN2", target_bir_lowering=False, debug=False, num_devices=NCORES
    )
    xT_ext = nc.declare_dram_parameter("xT", [BL, C, N], bf, isOutput=False)
    wqkvT_ext = nc.declare_dram_parameter("wqkvT", [C, 3 * C], bf, isOutput=False)
    wprojT_ext = nc.declare_dram_parameter("wprojT", [C, C], bf, isOutput=False)
    bproj_ext = nc.declare_dram_parameter("bproj", [1, C], bf, isOutput=False)
    out_ext = nc.declare_dram_parameter("out", [BL, N, C], f32, isOutput=True)

    CT = C // 128  # 6 input-channel tiles
    TT = N // 128  # 8 token tiles

    with tile.TileContext(nc) as tc, ExitStack() as ctx:
        const = ctx.enter_context(tc.tile_pool(name="const", bufs=1))
        xt_pool = ctx.enter_context(tc.tile_pool(name="xt", bufs=2 * CT))
        qk_pool = ctx.enter_context(tc.tile_pool(name="qk", bufs=24))
        va_pool = ctx.enter_context(tc.tile_pool(name="va", bufs=2 * TT))
        aoT_pool = ctx.enter_context(tc.tile_pool(name="aoT", bufs=12))
        aoU_pool = ctx.enter_context(tc.tile_pool(name="aoU", bufs=4))
        p_pool = ctx.enter_context(tc.tile_pool(name="pp", bufs=3))
        eps_pool = ctx.enter_context(tc.tile_pool(name="eps", bufs=2))
        osb_pool = ctx.enter_context(tc.tile_pool(name="osb", bufs=2))
        # PSUM budget (8 banks): st 4 + pv 2 + lin 1 + bc 1.
        # psA holds the [128,512] ST tiles (deep pipeline so ACT never
        # starves) and doubles as the psum pool for startup/tail linear
        # groups; psLIN (1 buf) serves the linear groups interleaved into
        # the attention loop.
        psA = ctx.enter_context(tc.tile_pool(name="psA", bufs=4, space="PSUM"))
        psPV = ctx.enter_context(tc.tile_pool(name="psPV", bufs=2, space="PSUM"))
        psLIN = ctx.enter_context(tc.tile_pool(name="psLIN", bufs=1, space="PSUM"))
        psBC = ctx.enter_context(tc.tile_pool(name="psBC", bufs=1, space="PSUM"))

        # --- constants ---
        # chunk the w_qkv DMAs by q/k/v feature block so the first qkT
        # matmul groups only wait on the chunk they read
        wq = []
        for i in range(CT):
            wt = const.tile([128, 3 * C], bf, name=f"wq{i}")
            for blk in range(3):
                nc.sync.dma_start(
                    wt[:, ds(blk * C, C)],
                    wqkvT_ext[ds(i * 128, 128), ds(blk * C, C)],
                )
            wq.append(wt)
        wp = []
        for i in range(CT):
            wt = const.tile([128, C], bf, name=f"wp{i}")
            nc.sync.dma_start(wt[:], wprojT_ext[ds(i * 128, 128), :])
            wp.append(wt)
        bpr = const.tile([1, C], bf, name="bpr")
        nc.sync.dma_start(bpr[:], bproj_ext[:])
        ones_tok = const.tile([1, 128], bf, name="ones_tok")
        nc.vector.memset(ones_tok[:], 1.0)
        ones64 = const.tile([1, 64], bf, name="ones64")
        nc.vector.memset(ones64[:], 1.0)

        # per-batch persistent tiles
        xt = {}
        qk = {}
        va = {}
        aoT = {}
        for b in range(BL):
            xt[b] = [
                xt_pool.tile([128, N], bf, tag="xt", name=f"xt{b}_{i}")
                for i in range(CT)
            ]
            qk[b] = [
                qk_pool.tile([128, N], bf, tag="qk", name=f"qk{b}_{f}")
                for f in range(12)
            ]
            va[b] = [
                va_pool.tile([128, H, 65], bf, tag="va", name=f"va{b}_{t}")
                for t in range(TT)
            ]
            aoT[b] = [
                aoT_pool.tile([128, N], bf, tag="aoT", name=f"aoT{b}_{i}")
                for i in range(CT)
            ]

        def load_xt(b):
            for i in range(CT):
                for hf in range(2):
                    nc.sync.dma_start(
                        xt[b][i][:, ds(hf * 512, 512)],
                        xT_ext[b, ds(i * 128, 128), ds(hf * 512, 512)],
                    )

        def qkT_group(b, ft, nt, pool=None):
            pool = pool or psLIN
            ps = pool.tile(
                [128, 512], f32, tag="st" if pool is psA else "lin",
                name=f"psqk{b}_{ft}_{nt}",
            )
            for ci in range(CT):
                nc.tensor.matmul(
                    ps[:],
                    lhsT=wq[ci][:, ds(ft * 128, 128)],
                    rhs=xt[b][ci][:, ds(nt * 512, 512)],
                    start=(ci == 0),
                    stop=(ci == CT - 1),
                )
            nc.vector.tensor_copy(qk[b][ft][:, ds(nt * 512, 512)], ps[:])

        def v_group(b, tt, pool=None):
            pool = pool or psLIN
            tg = "st" if pool is psA else "lin"
            ps0 = pool.tile([128, 512], f32, tag=tg, name=f"psv{b}_{tt}a")
            ps1 = pool.tile([128, 256], f32, tag=tg, name=f"psv{b}_{tt}b")
            for ci in range(CT):
                nc.tensor.matmul(
                    ps0[:],
                    lhsT=xt[b][ci][:, ds(tt * 128, 128)],
                    rhs=wq[ci][:, ds(2 * C, 512)],
                    start=(ci == 0),
                    stop=(ci == CT - 1),
                )
                nc.tensor.matmul(
                    ps1[:],
                    lhsT=xt[b][ci][:, ds(tt * 128, 128)],
                    rhs=wq[ci][:, ds(2 * C + 512, 256)],
                    start=(ci == 0),
                    stop=(ci == CT - 1),
                )
            nc.vector.memset(va[b][tt][:, :, ds(64, 1)], 1.0)
            nc.vector.tensor_copy(
                va[b][tt][:, ds(0, 8), ds(0, 64)],
                ps0[:].rearrange("p (h d) -> p h d", d=64),
            )
            nc.vector.tensor_copy(
                va[b][tt][:, ds(8, 4), ds(0, 64)],
                ps1[:].rearrange("p (h d) -> p h d", d=64),
            )

        def proj_group(b, tt, pool=None):
            pool = pool or psLIN
            tg = "st" if pool is psA else "lin"
            ps0 = pool.tile([128, 512], f32, tag=tg, name=f"pso{b}_{tt}a")
            ps1 = pool.tile([128, 256], f32, tag=tg, name=f"pso{b}_{tt}b")
            for ci in range(CT):
                nc.tensor.matmul(
                    ps0[:],
                    lhsT=aoT[b][ci][:, ds(tt * 128, 128)],
                    rhs=wp[ci][:, ds(0, 512)],
                    start=(ci == 0),
                    stop=False,
                )
                nc.tensor.matmul(
                    ps1[:],
                    lhsT=aoT[b][ci][:, ds(tt * 128, 128)],
                    rhs=wp[ci][:, ds(512, 256)],
                    start=(ci == 0),
                    stop=False,
                )
            nc.tensor.matmul(
                ps0[:], lhsT=ones_tok[:], rhs=bpr[:, ds(0, 512)],
                start=False, stop=True,
            )
            nc.tensor.matmul(
                ps1[:], lhsT=ones_tok[:], rhs=bpr[:, ds(512, 256)],
                start=False, stop=True,
            )
            osb = osb_pool.tile([128, C], f32, tag="osb", name=f"osb{b}_{tt}")
            nc.vector.tensor_copy(osb[:, ds(0, 512)], ps0[:])
            nc.vector.tensor_copy(osb[:, ds(512, 256)], ps1[:])
            nc.sync.dma_start(out_ext[b, ds(tt * 128, 128), :], osb[:])

        pending = deque()

        def drain(k):
            for _ in range(min(k, len(pending))):
                pending.popleft()()

        def attn_kt(b, h, st8, kt, pv):
            # one k-tile step of head h: ST matmuls, exp, PV accumulate
            q_tile = qk[b][h // 2]
            k_tile = qk[b][6 + h // 2]
            row = (h % 2) * 64
            st = [
                psA.tile([128, 512], f32, tag="st", name=f"st{b}_{h}_{kt}_{qc}")
                for qc in range(2)
            ]
            for qc in range(2):
                nc.tensor.matmul(
                    st[qc][:],
                    lhsT=k_tile[ds(row, 64), ds(kt * 128, 128)],
                    rhs=q_tile[ds(row, 64), ds(qc * 512, 512)],
                    start=True,
                    stop=True,
                )
            pt = p_pool.tile([128, N], bf, tag="pt", name=f"pt{b}_{h}_{kt}")
            for qc in range(2):
                nc.scalar.activation(
                    pt[:, ds(qc * 512, 512)], st[qc][:], Exp, scale=SCALE
                )
            for qc in range(2):
                nc.tensor.matmul(
                    pv[qc][:],
                    lhsT=va[b][kt][:, h, :],
                    rhs=pt[:, ds(qc * 512, 512)],
                    start=(kt == 0),
                    stop=(kt == TT - 1),
                )

        def head_start(b, h):
            # allocate this head's state and emit its first k-tile step so
            # ACT has work queued across the previous head's epilogue
            pv = [
                psPV.tile([65, 512], f32, tag="pv", name=f"pv{b}_{h}_{qc}")
                for qc in range(2)
            ]
            attn_kt(b, h, None, 0, pv)
            return pv

        def head_rest(b, h, pv):
            for kt in range(1, TT):
                attn_kt(b, h, None, kt, pv)
                if kt == 3:
                    drain(1)
            aoU = aoU_pool.tile([64, N], bf, tag="aoU", name=f"aoU{b}_{h}")
            den = eps_pool.tile([1, N], f32, tag="den", name=f"den{b}_{h}")
            for qc in range(2):
                nc.vector.tensor_copy(
                    aoU[:, ds(qc * 512, 512)], pv[qc][ds(0, 64), :]
                )
                nc.vector.tensor_copy(
                    den[:, ds(qc * 512, 512)], pv[qc][ds(64, 1), :]
                )
            return aoU, den

        def head_epilogue(b, h, aoU, den):
            nc.vector.reciprocal_approx_fast(den[:], den[:])
            recb = eps_pool.tile([1, N], bf, tag="recb", name=f"recb{b}_{h}")
            nc.vector.tensor_copy(recb[:], den[:])
            row = (h % 2) * 64
            ao_tile = aoT[b][h // 2]
            for hf in range(2):
                bc = psBC.tile([64, 512], f32, tag="bc", name=f"bc{b}_{h}_{hf}")
                nc.tensor.matmul(
                    bc[:], lhsT=ones64[:], rhs=recb[:, ds(hf * 512, 512)],
                    start=True, stop=True,
                )
                nc.vector.tensor_mul(
                    ao_tile[ds(row, 64), ds(hf * 512, 512)],
                    aoU[:, ds(hf * 512, 512)],
                    bc[:],
                )

        # --- schedule ---
        # startup: only what head 0 needs up front (q/k tiles ft0+ft6, all
        # of V); the remaining qkT groups of batch 0 are interleaved into
        # the early attention heads, ordered so head h's tiles are ready
        # ~2 heads ahead of their first use.
        load_xt(0)
        for ft in (0, 6):
            for nt in range(2):
                qkT_group(0, ft, nt, pool=psA)
        for tt in range(TT):
            v_group(0, tt, pool=psA)
        for ft_pair in range(1, 6):
            for ft in (ft_pair, 6 + ft_pair):
                for nt in range(2):
                    pending.append(lambda ft=ft, nt=nt: qkT_group(0, ft, nt))

        for b in range(BL):
            if b + 1 < BL:
                load_xt(b + 1)
                # order for batch b+1's head 0: ft0+ft6 first, then all of
                # V, then the remaining ft pairs in head-use order
                for ft in (0, 6):
                    for nt in range(2):
                        pending.append(
                            lambda b=b + 1, ft=ft, nt=nt: qkT_group(b, ft, nt)
                        )
                for tt in range(TT):
                    pending.append(lambda b=b + 1, tt=tt: v_group(b, tt))
                for ft_pair in range(1, 6):
                    for ft in (ft_pair, 6 + ft_pair):
                        for nt in range(2):
                            pending.append(
                                lambda b=b + 1, ft=ft, nt=nt: qkT_group(b, ft, nt)
                            )
            # software-pipelined head loop: the next head's first k-tile is
            # emitted before the current head's epilogue so ACT never idles
            # across head boundaries
            pv_cur = head_start(b, 0)
            for h in range(H):
                aoU, den = head_rest(b, h, pv_cur)
                if h + 1 < H:
                    pv_cur = head_start(b, h + 1)
                head_epilogue(b, h, aoU, den)
                drain(2)
            if b == BL - 1:
                drain(len(pending))
                # pipelined tail: alternate psum pools so copy-out of one
                # proj group overlaps the matmuls of the next
                for tt in range(TT):
                    proj_group(b, tt, pool=(psA if tt % 2 == 0 else psLIN))
            else:
                for tt in range(TT):
                    pending.append(lambda b=b, tt=tt: proj_group(b, tt))

    nc.finalize()
    return nc


_GRAPH = None
LAST_EXEC_TIME_NS = None
LAST_RESULTS = None


def kernel(x, w_qkv, w_proj, b_proj):
    global _GRAPH, LAST_EXEC_TIME_NS, LAST_RESULTS
    import os
    from concourse.bass_utils import run_bass_kernel_spmd

    x = np.asarray(x, dtype=np.float32)
    w_qkv = np.asarray(w_qkv, dtype=np.float32)
    w_proj = np.asarray(w_proj, dtype=np.float32)
    b_proj = np.asarray(b_proj, dtype=np.float32)

    # shard: batches 2i, 2i+1 -> core i; pre-transpose x to [BL, C, N]
    xT = np.ascontiguousarray(
        x.reshape(NCORES, BL, N, C).transpose(0, 1, 3, 2)
    ).astype(BF16)
    wqkvT = np.ascontiguousarray(w_qkv.T).astype(BF16)
    wprojT = np.ascontiguousarray(w_proj.T).astype(BF16)
    bp = np.ascontiguousarray(b_proj.reshape(1, C)).astype(BF16)

    if _GRAPH is None:
        _GRAPH = _build_graph()

    in_maps = [
        {"xT": xT[i], "wqkvT": wqkvT, "wprojT": wprojT, "bproj": bp}
        for i in range(NCORES)
    ]
    trace = os.environ.get("BASS_KERNEL_TRACE") == "1"
    tmpdir = os.environ.get("BASS_KERNEL_TRACE_DIR") if trace else None
    if tmpdir:
        import shutil

        shutil.rmtree(tmpdir, ignore_errors=True)
        os.makedirs(tmpdir, exist_ok=True)
    res = run_bass_kernel_spmd(
        _GRAPH, in_maps, core_ids=list(range(NCORES)), trace=trace, tmpdir=tmpdir
    )
    LAST_EXEC_TIME_NS = res.exec_time_ns
    LAST_RESULTS = res
    out = np.concatenate([res.results[i]["out"] for i in range(NCORES)], axis=0)
    return out.astype(np.float32)


# revision 39
# speedup vs baseline: 1.0108x; 1.0108x over previous
"""Multi-head attention (B=16, N=1024, C=768, H=12) on 8 TRN2 NeuronCores.

Strategy: pure data-parallel over batch (2 batches per core, no collectives).
All matmuls run in bf16 (1 PE cycle/row vs 4 for fp32; rel err ~6e-3).

Per-core pipeline, per batch b (layouts chosen so no transposes are needed):
  1. qkT  [1536, 1024]  = w_qkv[0:1536] @ x[b].T        (feature-major Q,K)
  2. vaug [1024, 12*65] = x[b] @ w_qkv[1536:].T         (token-major V, with
     a ones-column per head -> softmax denominators fall out of the PV matmul)
  3. per head h: S.T = kT.T @ qT (PE), P = exp(S.T * scale) (ACT, no
     max-subtraction needed: logits ~ N(0,1)), PV: outT[65, q] = vaug.T @ P
     accumulated over k tiles.  Row 64 of PV psum = softmax denominator.
     Normalize: reciprocal_approx_fast on the denom row, bf16 cast,
     broadcast across 64 partitions via a K=1 ones matmul, multiply.
  4. proj: out[tok, 768] = attn_outT.T @ w_proj.T + bias (bias folded into
     the matmul as an extra K=1 ones row).

PE/ACT overlap: attention is ACT(exp)-bound, so the projection matmul groups
of the previous batch and the QKV matmul groups of the next batch are
interleaved into the attention head loop via a pending-work queue.
"""

from collections import deque

import numpy as np
import ml_dtypes

B, N, C = 16, 1024, 768
H, HD = 12, 64
NCORES = 8
BL = B // NCORES  # batches per core
SCALE = HD ** -0.5

BF16 = ml_dtypes.bfloat16


def _build_graph():
    import concourse.mybir as mybir
    import concourse.tile as tile
    from concourse import bacc
    from concourse.bass import ds
    from contextlib import ExitStack

    bf = mybir.dt.bfloat16
    f32 = mybir.dt.float32
    Exp = mybir.ActivationFunctionType.Exp

    nc = bacc.Bacc(
        "TRN2", target_bir_lowering=False, debug=False, num_devices=NCORES
    )
    xT_ext = nc.declare_dram_parameter("xT", [BL, C, N], bf, isOutput=False)
    wqkvT_ext = nc.declare_dram_parameter("wqkvT", [C, 3 * C], bf, isOutput=False)
    wprojT_ext = nc.declare_dram_parameter("wprojT", [C, C], bf, isOutput=False)
    bproj_ext = nc.declare_dram_parameter("bproj", [1, C], bf, isOutput=False)
    out_ext = nc.declare_dram_parameter("out", [BL, N, C], f32, isOutput=True)

    CT = C // 128  # 6 input-channel tiles
    TT = N // 128  # 8 token tiles

    with tile.TileContext(nc) as tc, ExitStack() as ctx:
        const = ctx.enter_context(tc.tile_pool(name="const", bufs=1))
        xt_pool = ctx.enter_context(tc.tile_pool(name="xt", bufs=2 * CT))
        qk_pool = ctx.enter_context(tc.tile_pool(name="qk", bufs=24))
        va_pool = ctx.enter_context(tc.tile_pool(name="va", bufs=2 * TT))
        aoT_pool = ctx.enter_context(tc.tile_pool(name="aoT", bufs=12))
        aoU_pool = ctx.enter_context(tc.tile_pool(name="aoU", bufs=4))
        p_pool = ctx.enter_context(tc.tile_pool(name="pp", bufs=3))
        eps_pool = ctx.enter_context(tc.tile_pool(name="eps", bufs=2))
        osb_pool = ctx.enter_context(tc.tile_pool(name="osb", bufs=2))
        # PSUM budget (8 banks): st 4 + pv 2 + lin 1 + bc 1.
        # psA holds the [128,512] ST tiles (deep pipeline so ACT never
        # starves) and doubles as the psum pool for startup/tail linear
        # groups; psLIN (1 buf) serves the linear groups interleaved into
        # the attention loop.
        psA = ctx.enter_context(tc.tile_pool(name="psA", bufs=4, space="PSUM"))
        psPV = ctx.enter_context(tc.tile_pool(name="psPV", bufs=2, space="PSUM"))
        psLIN = ctx.enter_context(tc.tile_pool(name="psLIN", bufs=1, space="PSUM"))
        psBC = ctx.enter_context(tc.tile_pool(name="psBC", bufs=1, space="PSUM"))

        # --- constants ---
        # chunk the w_qkv DMAs by q/k/v feature block so the first qkT
        # matmul groups only wait on the chunk they read
        wq = []
        for i in range(CT):
            wt = const.tile([128, 3 * C], bf, name=f"wq{i}")
            for blk in range(3):
                nc.sync.dma_start(
                    wt[:, ds(blk * C, C)],
                    wqkvT_ext[ds(i * 128, 128), ds(blk * C, C)],
                )
            wq.append(wt)
        wp = []
        for i in range(CT):
            wt = const.tile([128, C], bf, name=f"wp{i}")
            nc.sync.dma_start(wt[:], wprojT_ext[ds(i * 128, 128), :])
            wp.append(wt)
        bpr = const.tile([1, C], bf, name="bpr")
        nc.sync.dma_start(bpr[:], bproj_ext[:])
        ones_tok = const.tile([1, 128], bf, name="ones_tok")
        nc.vector.memset(ones_tok[:], 1.0)
        ones64 = const.tile([1, 64], bf, name="ones64")
        nc.vector.memset(ones64[:], 1.0)

        # per-batch persistent tiles
        xt = {}
        qk = {}
        va = {}
        aoT = {}
        for b in range(BL):
            xt[b] = [
                xt_pool.tile([128, N], bf, tag="xt", name=f"xt{b}_{i}")
                for i in range(CT)
            ]
            qk[b] = [
                qk_pool.tile([128, N], bf, tag="qk", name=f"qk{b}_{f}")
                for f in range(12)
            ]
            va[b] = [
                va_pool.tile([128, H, 65], bf, tag="va", name=f"va{b}_{t}")
                for t in range(TT)
            ]
            aoT[b] = [
                aoT_pool.tile([128, N], bf, tag="aoT", name=f"aoT{b}_{i}")
                for i in range(CT)
            ]

        def load_xt(b):
            for i in range(CT):
                nc.sync.dma_start(xt[b][i][:], xT_ext[b, ds(i * 128, 128), :])

        def qkT_group(b, ft, nt, pool=None):
            pool = pool or psLIN
            ps = pool.tile(
                [128, 512], f32, tag="st" if pool is psA else "lin",
                name=f"psqk{b}_{ft}_{nt}",
            )
            for ci in range(CT):
                nc.tensor.matmul(
                    ps[:],
                    lhsT=wq[ci][:, ds(ft * 128, 128)],
                    rhs=xt[b][ci][:, ds(nt * 512, 512)],
                    start=(ci == 0),
                    stop=(ci == CT - 1),
                )
            nc.vector.tensor_copy(qk[b][ft][:, ds(nt * 512, 512)], ps[:])

        def v_group(b, tt, pool=None):
            pool = pool or psLIN
            tg = "st" if pool is psA else "lin"
            ps0 = pool.tile([128, 512], f32, tag=tg, name=f"psv{b}_{tt}a")
            ps1 = pool.tile([128, 256], f32, tag=tg, name=f"psv{b}_{tt}b")
            for ci in range(CT):
                nc.tensor.matmul(
                    ps0[:],
                    lhsT=xt[b][ci][:, ds(tt * 128, 128)],
                    rhs=wq[ci][:, ds(2 * C, 512)],
                    start=(ci == 0),
                    stop=(ci == CT - 1),
                )
                nc.tensor.matmul(
                    ps1[:],
                    lhsT=xt[b][ci][:, ds(tt * 128, 128)],
                    rhs=wq[ci][:, ds(2 * C + 512, 256)],
                    start=(ci == 0),
                    stop=(ci == CT - 1),
                )
            nc.vector.memset(va[b][tt][:, :, ds(64, 1)], 1.0)
            nc.vector.tensor_copy(
                va[b][tt][:, ds(0, 8), ds(0, 64)],
                ps0[:].rearrange("p (h d) -> p h d", d=64),
            )
            nc.vector.tensor_copy(
                va[b][tt][:, ds(8, 4), ds(0, 64)],
                ps1[:].rearrange("p (h d) -> p h d", d=64),
            )

        def proj_group(b, tt, pool=None):
            pool = pool or psLIN
            tg = "st" if pool is psA else "lin"
            ps0 = pool.tile([128, 512], f32, tag=tg, name=f"pso{b}_{tt}a")
            ps1 = pool.tile([128, 256], f32, tag=tg, name=f"pso{b}_{tt}b")
            for ci in range(CT):
                nc.tensor.matmul(
                    ps0[:],
                    lhsT=aoT[b][ci][:, ds(tt * 128, 128)],
                    rhs=wp[ci][:, ds(0, 512)],
                    start=(ci == 0),
                    stop=False,
                )
                nc.tensor.matmul(
                    ps1[:],
                    lhsT=aoT[b][ci][:, ds(tt * 128, 128)],
                    rhs=wp[ci][:, ds(512, 256)],
                    start=(ci == 0),
                    stop=False,
                )
            nc.tensor.matmul(
                ps0[:], lhsT=ones_tok[:], rhs=bpr[:, ds(0, 512)],
                start=False, stop=True,
            )
            nc.tensor.matmul(
                ps1[:], lhsT=ones_tok[:], rhs=bpr[:, ds(512, 256)],
                start=False, stop=True,
            )
            osb = osb_pool.tile([128, C], f32, tag="osb", name=f"osb{b}_{tt}")
            nc.vector.tensor_copy(osb[:, ds(0, 512)], ps0[:])
            nc.vector.tensor_copy(osb[:, ds(512, 256)], ps1[:])
            nc.sync.dma_start(out_ext[b, ds(tt * 128, 128), :], osb[:])

        pending = deque()

        def drain(k):
            for _ in range(min(k, len(pending))):
                pending.popleft()()

        def attn_kt(b, h, st8, kt, pv):
            # one k-tile step of head h: ST matmuls, exp, PV accumulate
            q_tile = qk[b][h // 2]
            k_tile = qk[b][6 + h // 2]
            row = (h % 2) * 64
            st = [
                psA.tile([128, 512], f32, tag="st", name=f"st{b}_{h}_{kt}_{qc}")
                for qc in range(2)
            ]
            for qc in range(2):
                nc.tensor.matmul(
                    st[qc][:],
                    lhsT=k_tile[ds(row, 64), ds(kt * 128, 128)],
                    rhs=q_tile[ds(row, 64), ds(qc * 512, 512)],
                    start=True,
                    stop=True,
                )
            pt = p_pool.tile([128, N], bf, tag="pt", name=f"pt{b}_{h}_{kt}")
            for qc in range(2):
                nc.scalar.activation(
                    pt[:, ds(qc * 512, 512)], st[qc][:], Exp, scale=SCALE
                )
            for qc in range(2):
                nc.tensor.matmul(
                    pv[qc][:],
                    lhsT=va[b][kt][:, h, :],
                    rhs=pt[:, ds(qc * 512, 512)],
                    start=(kt == 0),
                    stop=(kt == TT - 1),
                )

        def head_start(b, h):
            # allocate this head's state and emit its first k-tile step so
            # ACT has work queued across the previous head's epilogue
            pv = [
                psPV.tile([65, 512], f32, tag="pv", name=f"pv{b}_{h}_{qc}")
                for qc in range(2)
            ]
            attn_kt(b, h, None, 0, pv)
            return pv

        def head_rest(b, h, pv):
            for kt in range(1, TT):
                attn_kt(b, h, None, kt, pv)
                if kt == 3:
                    drain(1)
            aoU = aoU_pool.tile([64, N], bf, tag="aoU", name=f"aoU{b}_{h}")
            den = eps_pool.tile([1, N], f32, tag="den", name=f"den{b}_{h}")
            for qc in range(2):
                nc.vector.tensor_copy(
                    aoU[:, ds(qc * 512, 512)], pv[qc][ds(0, 64), :]
                )
                nc.vector.tensor_copy(
                    den[:, ds(qc * 512, 512)], pv[qc][ds(64, 1), :]
                )
            return aoU, den

        def head_epilogue(b, h, aoU, den):
            nc.vector.reciprocal_approx_fast(den[:], den[:])
            recb = eps_pool.tile([1, N], bf, tag="recb", name=f"recb{b}_{h}")
            nc.vector.tensor_copy(recb[:], den[:])
            row = (h % 2) * 64
            ao_tile = aoT[b][h // 2]
            for hf in range(2):
                bc = psBC.tile([64, 512], f32, tag="bc", name=f"bc{b}_{h}_{hf}")
                nc.tensor.matmul(
                    bc[:], lhsT=ones64[:], rhs=recb[:, ds(hf * 512, 512)],
                    start=True, stop=True,
                )
                nc.vector.tensor_mul(
                    ao_tile[ds(row, 64), ds(hf * 512, 512)],
                    aoU[:, ds(hf * 512, 512)],
                    bc[:],
                )

        # --- schedule ---
        # startup: only what head 0 needs up front (q/k tiles ft0+ft6, all
        # of V); the remaining qkT groups of batch 0 are interleaved into
        # the early attention heads, ordered so head h's tiles are ready
        # ~2 heads ahead of their first use.
        load_xt(0)
        for ft in (0, 6):
            for nt in range(2):
                qkT_group(0, ft, nt, pool=psA)
        for tt in range(TT):
            v_group(0, tt, pool=psA)
        for ft_pair in range(1, 6):
            for ft in (ft_pair, 6 + ft_pair):
                for nt in range(2):
                    pending.append(lambda ft=ft, nt=nt: qkT_group(0, ft, nt))

        for b in range(BL):
            if b + 1 < BL:
                load_xt(b + 1)
                # order for batch b+1's head 0: ft0+ft6 first, then all of
                # V, then the remaining ft pairs in head-use order
                for ft in (0, 6):
                    for nt in range(2):
                        pending.append(
                            lambda b=b + 1, ft=ft, nt=nt: qkT_group(b, ft, nt)
                        )
                for tt in range(TT):
                    pending.append(lambda b=b + 1, tt=tt: v_group(b, tt))
                for ft_pair in range(1, 6):
                    for ft in (ft_pair, 6 + ft_pair):
                        for nt in range(2):
                            pending.append(
                                lambda b=b + 1, ft=ft, nt=nt: qkT_group(b, ft, nt)
                            )
            # software-pipelined head loop: the next head's first k-tile is
            # emitted before the current head's epilogue so ACT never idles
            # across head boundaries
            pv_cur = head_start(b, 0)
            for h in range(H):
                aoU, den = head_rest(b, h, pv_cur)
                if h + 1 < H:
                    pv_cur = head_start(b, h + 1)
                head_epilogue(b, h, aoU, den)
                drain(2)
            if b == BL - 1:
                drain(len(pending))
                # pipelined tail: alternate psum pools so copy-out of one
                # proj group overlaps the matmuls of the next
                for tt in range(TT):
                    proj_group(b, tt, pool=(psA if tt % 2 == 0 else psLIN))
            else:
                for tt in range(TT):
                    pending.append(lambda b=b, tt=tt: proj_group(b, tt))

    nc.finalize()
    return nc


_GRAPH = None
LAST_EXEC_TIME_NS = None
LAST_RESULTS = None


def kernel(x, w_qkv, w_proj, b_proj):
    global _GRAPH, LAST_EXEC_TIME_NS, LAST_RESULTS
    import os
    from concourse.bass_utils import run_bass_kernel_spmd

    x = np.asarray(x, dtype=np.float32)
    w_qkv = np.asarray(w_qkv, dtype=np.float32)
    w_proj = np.asarray(w_proj, dtype=np.float32)
    b_proj = np.asarray(b_proj, dtype=np.float32)

    # shard: batches 2i, 2i+1 -> core i; pre-transpose x to [BL, C, N]
    xT = np.ascontiguousarray(
        x.reshape(NCORES, BL, N, C).transpose(0, 1, 3, 2)
    ).astype(BF16)
    wqkvT = np.ascontiguousarray(w_qkv.T).astype(BF16)
    wprojT = np.ascontiguousarray(w_proj.T).astype(BF16)
    bp = np.ascontiguousarray(b_proj.reshape(1, C)).astype(BF16)

    if _GRAPH is None:
        _GRAPH = _build_graph()

    in_maps = [
        {"xT": xT[i], "wqkvT": wqkvT, "wprojT": wprojT, "bproj": bp}
        for i in range(NCORES)
    ]
    trace = os.environ.get("BASS_KERNEL_TRACE") == "1"
    tmpdir = os.environ.get("BASS_KERNEL_TRACE_DIR") if trace else None
    if tmpdir:
        import shutil

        shutil.rmtree(tmpdir, ignore_errors=True)
        os.makedirs(tmpdir, exist_ok=True)
    res = run_bass_kernel_spmd(
        _GRAPH, in_maps, core_ids=list(range(NCORES)), trace=trace, tmpdir=tmpdir
    )
    LAST_EXEC_TIME_NS = res.exec_time_ns
    LAST_RESULTS = res
    out = np.concatenate([res.results[i]["out"] for i in range(NCORES)], axis=0)
    return out.astype(np.float32)


# revision 40
# speedup vs baseline: 1.0149x; 1.0041x over previous
"""Multi-head attention (B=16, N=1024, C=768, H=12) on 8 TRN2 NeuronCores.

Strategy: pure data-parallel over batch (2 batches per core, no collectives).
All matmuls run in bf16 (1 PE cycle/row vs 4 for fp32; rel err ~6e-3).

Per-core pipeline, per batch b (layouts chosen so no transposes are needed):
  1. qkT  [1536, 1024]  = w_qkv[0:1536] @ x[b].T        (feature-major Q,K)
  2. vaug [1024, 12*65] = x[b] @ w_qkv[1536:].T         (token-major V, with
     a ones-column per head -> softmax denominators fall out of the PV matmul)
  3. per head h: S.T = kT.T @ qT (PE), P = exp(S.T * scale) (ACT, no
     max-subtraction needed: logits ~ N(0,1)), PV: outT[65, q] = vaug.T @ P
     accumulated over k tiles.  Row 64 of PV psum = softmax denominator.
     Normalize: reciprocal_approx_fast on the denom row, bf16 cast,
     broadcast across 64 partitions via a K=1 ones matmul, multiply.
  4. proj: out[tok, 768] = attn_outT.T @ w_proj.T + bias (bias folded into
     the matmul as an extra K=1 ones row).

PE/ACT overlap: attention is ACT(exp)-bound, so the projection matmul groups
of the previous batch and the QKV matmul groups of the next batch are
interleaved into the attention head loop via a pending-work queue.
"""

from collections import deque

import numpy as np
import ml_dtypes

B, N, C = 16, 1024, 768
H, HD = 12, 64
NCORES = 8
BL = B // NCORES  # batches per core
SCALE = HD ** -0.5

BF16 = ml_dtypes.bfloat16


def _build_graph():
    import concourse.mybir as mybir
    import concourse.tile as tile
    from concourse import bacc
    from concourse.bass import ds
    from contextlib import ExitStack

    bf = mybir.dt.bfloat16
    f32 = mybir.dt.float32
    Exp = mybir.ActivationFunctionType.Exp

    nc = bacc.Bacc(
        "TRN2", target_bir_lowering=False, debug=False, num_devices=NCORES
    )
    xT_ext = nc.declare_dram_parameter("xT", [BL, C, N], bf, isOutput=False)
    wqkvT_ext = nc.declare_dram_parameter("wqkvT", [C, 3 * C], bf, isOutput=False)
    wprojT_ext = nc.declare_dram_parameter("wprojT", [C, C], bf, isOutput=False)
    bproj_ext = nc.declare_dram_parameter("bproj", [1, C], bf, isOutput=False)
    out_ext = nc.declare_dram_parameter("out", [BL, N, C], f32, isOutput=True)

    CT = C // 128  # 6 input-channel tiles
    TT = N // 128  # 8 token tiles

    with tile.TileContext(nc) as tc, ExitStack() as ctx:
        const = ctx.enter_context(tc.tile_pool(name="const", bufs=1))
        xt_pool = ctx.enter_context(tc.tile_pool(name="xt", bufs=2 * CT))
        qk_pool = ctx.enter_context(tc.tile_pool(name="qk", bufs=24))
        va_pool = ctx.enter_context(tc.tile_pool(name="va", bufs=2 * TT))
        aoT_pool = ctx.enter_context(tc.tile_pool(name="aoT", bufs=12))
        aoU_pool = ctx.enter_context(tc.tile_pool(name="aoU", bufs=4))
        p_pool = ctx.enter_context(tc.tile_pool(name="pp", bufs=3))
        eps_pool = ctx.enter_context(tc.tile_pool(name="eps", bufs=2))
        osb_pool = ctx.enter_context(tc.tile_pool(name="osb", bufs=2))
        # PSUM budget (8 banks): st 4 + pv 2 + lin 1 + bc 1.
        # psA holds the [128,512] ST tiles (deep pipeline so ACT never
        # starves) and doubles as the psum pool for startup/tail linear
        # groups; psLIN (1 buf) serves the linear groups interleaved into
        # the attention loop.
        psA = ctx.enter_context(tc.tile_pool(name="psA", bufs=4, space="PSUM"))
        psPV = ctx.enter_context(tc.tile_pool(name="psPV", bufs=2, space="PSUM"))
        psLIN = ctx.enter_context(tc.tile_pool(name="psLIN", bufs=1, space="PSUM"))
        psBC = ctx.enter_context(tc.tile_pool(name="psBC", bufs=1, space="PSUM"))

        # --- constants ---
        # chunk the w_qkv DMAs by q/k/v feature block so the first qkT
        # matmul groups only wait on the chunk they read
        wq = []
        for i in range(CT):
            wt = const.tile([128, 3 * C], bf, name=f"wq{i}")
            for blk in range(3):
                nc.sync.dma_start(
                    wt[:, ds(blk * C, C)],
                    wqkvT_ext[ds(i * 128, 128), ds(blk * C, C)],
                )
            wq.append(wt)
        wp = []
        for i in range(CT):
            wt = const.tile([128, C], bf, name=f"wp{i}")
            nc.sync.dma_start(wt[:], wprojT_ext[ds(i * 128, 128), :])
            wp.append(wt)
        bpr = const.tile([1, C], bf, name="bpr")
        nc.sync.dma_start(bpr[:], bproj_ext[:])
        ones_tok = const.tile([1, 128], bf, name="ones_tok")
        nc.vector.memset(ones_tok[:], 1.0)
        ones64 = const.tile([1, 64], bf, name="ones64")
        nc.vector.memset(ones64[:], 1.0)

        # per-batch persistent tiles
        xt = {}
        qk = {}
        va = {}
        aoT = {}
        for b in range(BL):
            xt[b] = [
                xt_pool.tile([128, N], bf, tag="xt", name=f"xt{b}_{i}")
                for i in range(CT)
            ]
            qk[b] = [
                qk_pool.tile([128, N], bf, tag="qk", name=f"qk{b}_{f}")
                for f in range(12)
            ]
            va[b] = [
                va_pool.tile([128, H, 65], bf, tag="va", name=f"va{b}_{t}")
                for t in range(TT)
            ]
            aoT[b] = [
                aoT_pool.tile([128, N], bf, tag="aoT", name=f"aoT{b}_{i}")
                for i in range(CT)
            ]

        def load_xt(b):
            for i in range(CT):
                nc.sync.dma_start(xt[b][i][:], xT_ext[b, ds(i * 128, 128), :])

        def qkT_group(b, ft, nt, pool=None):
            pool = pool or psLIN
            ps = pool.tile(
                [128, 512], f32, tag="st" if pool is psA else "lin",
                name=f"psqk{b}_{ft}_{nt}",
            )
            for ci in range(CT):
                nc.tensor.matmul(
                    ps[:],
                    lhsT=wq[ci][:, ds(ft * 128, 128)],
                    rhs=xt[b][ci][:, ds(nt * 512, 512)],
                    start=(ci == 0),
                    stop=(ci == CT - 1),
                )
            nc.vector.tensor_copy(qk[b][ft][:, ds(nt * 512, 512)], ps[:])

        def v_group(b, tt, pool=None):
            pool = pool or psLIN
            tg = "st" if pool is psA else "lin"
            ps0 = pool.tile([128, 512], f32, tag=tg, name=f"psv{b}_{tt}a")
            ps1 = pool.tile([128, 256], f32, tag=tg, name=f"psv{b}_{tt}b")
            for ci in range(CT):
                nc.tensor.matmul(
                    ps0[:],
                    lhsT=xt[b][ci][:, ds(tt * 128, 128)],
                    rhs=wq[ci][:, ds(2 * C, 512)],
                    start=(ci == 0),
                    stop=(ci == CT - 1),
                )
                nc.tensor.matmul(
                    ps1[:],
                    lhsT=xt[b][ci][:, ds(tt * 128, 128)],
                    rhs=wq[ci][:, ds(2 * C + 512, 256)],
                    start=(ci == 0),
                    stop=(ci == CT - 1),
                )
            nc.vector.memset(va[b][tt][:, :, ds(64, 1)], 1.0)
            nc.vector.tensor_copy(
                va[b][tt][:, ds(0, 8), ds(0, 64)],
                ps0[:].rearrange("p (h d) -> p h d", d=64),
            )
            nc.vector.tensor_copy(
                va[b][tt][:, ds(8, 4), ds(0, 64)],
                ps1[:].rearrange("p (h d) -> p h d", d=64),
            )

        def proj_group(b, tt, pool=None):
            pool = pool or psLIN
            tg = "st" if pool is psA else "lin"
            ps0 = pool.tile([128, 512], f32, tag=tg, name=f"pso{b}_{tt}a")
            ps1 = pool.tile([128, 256], f32, tag=tg, name=f"pso{b}_{tt}b")
            for ci in range(CT):
                nc.tensor.matmul(
                    ps0[:],
                    lhsT=aoT[b][ci][:, ds(tt * 128, 128)],
                    rhs=wp[ci][:, ds(0, 512)],
                    start=(ci == 0),
                    stop=False,
                )
                nc.tensor.matmul(
                    ps1[:],
                    lhsT=aoT[b][ci][:, ds(tt * 128, 128)],
                    rhs=wp[ci][:, ds(512, 256)],
                    start=(ci == 0),
                    stop=False,
                )
            nc.tensor.matmul(
                ps0[:], lhsT=ones_tok[:], rhs=bpr[:, ds(0, 512)],
                start=False, stop=True,
            )
            nc.tensor.matmul(
                ps1[:], lhsT=ones_tok[:], rhs=bpr[:, ds(512, 256)],
                start=False, stop=True,
            )
            osb = osb_pool.tile([128, C], f32, tag="osb", name=f"osb{b}_{tt}")
            nc.vector.tensor_copy(osb[:, ds(0, 512)], ps0[:])
            nc.vector.tensor_copy(osb[:, ds(512, 256)], ps1[:])
            nc.sync.dma_start(out_ext[b, ds(tt * 128, 128), :], osb[:])

        pending = deque()

        def drain(k):
            for _ in range(min(k, len(pending))):
                pending.popleft()()

        def attn_head(b, h):
            q_tile = qk[b][h // 2]
            k_tile = qk[b][6 + h // 2]
            row = (h % 2) * 64
            aoU = aoU_pool.tile([64, N], bf, tag="aoU", name=f"aoU{b}_{h}")
            den = eps_pool.tile([1, N], f32, tag="den", name=f"den{b}_{h}")
            pv = [
                psPV.tile([65, 512], f32, tag="pv", name=f"pv{b}_{h}_{qc}")
                for qc in range(2)
            ]
            for kt in range(TT):
                st = [
                    psA.tile([128, 512], f32, tag="st", name=f"st{b}_{h}_{kt}_{qc}")
                    for qc in range(2)
                ]
                for qc in range(2):
                    nc.tensor.matmul(
                        st[qc][:],
                        lhsT=k_tile[ds(row, 64), ds(kt * 128, 128)],
                        rhs=q_tile[ds(row, 64), ds(qc * 512, 512)],
                        start=True,
                        stop=True,
                    )
                pt = p_pool.tile([128, N], bf, tag="pt", name=f"pt{b}_{h}_{kt}")
                for qc in range(2):
                    nc.scalar.activation(
                        pt[:, ds(qc * 512, 512)], st[qc][:], Exp, scale=SCALE
                    )
                for qc in range(2):
                    nc.tensor.matmul(
                        pv[qc][:],
                        lhsT=va[b][kt][:, h, :],
                        rhs=pt[:, ds(qc * 512, 512)],
                        start=(kt == 0),
                        stop=(kt == TT - 1),
                    )
                if kt == 3:
                    drain(1)
            for qc in range(2):
                nc.vector.tensor_copy(
                    aoU[:, ds(qc * 512, 512)], pv[qc][ds(0, 64), :]
                )
                nc.vector.tensor_copy(
                    den[:, ds(qc * 512, 512)], pv[qc][ds(64, 1), :]
                )
            nc.vector.reciprocal_approx_fast(den[:], den[:])
            recb = eps_pool.tile([1, N], bf, tag="recb", name=f"recb{b}_{h}")
            nc.vector.tensor_copy(recb[:], den[:])
            ao_tile = aoT[b][h // 2]
            for hf in range(2):
                bc = psBC.tile([64, 512], f32, tag="bc", name=f"bc{b}_{h}_{hf}")
                nc.tensor.matmul(
                    bc[:], lhsT=ones64[:], rhs=recb[:, ds(hf * 512, 512)],
                    start=True, stop=True,
                )
                nc.vector.tensor_mul(
                    ao_tile[ds(row, 64), ds(hf * 512, 512)],
                    aoU[:, ds(hf * 512, 512)],
                    bc[:],
                )

        # --- schedule ---
        # startup: only what head 0 needs up front (q/k tiles ft0+ft6, all
        # of V); the remaining qkT groups of batch 0 are interleaved into
        # the early attention heads, ordered so head h's tiles are ready
        # ~2 heads ahead of their first use.
        load_xt(0)
        for ft in (0, 6):
            for nt in range(2):
                qkT_group(0, ft, nt, pool=psA)
        for tt in range(TT):
            v_group(0, tt, pool=psA)
        for ft_pair in range(1, 6):
            for ft in (ft_pair, 6 + ft_pair):
                for nt in range(2):
                    pending.append(lambda ft=ft, nt=nt: qkT_group(0, ft, nt))

        for b in range(BL):
            if b + 1 < BL:
                load_xt(b + 1)
                # order for batch b+1's head 0: ft0+ft6 first, then all of
                # V, then the remaining ft pairs in head-use order
                for ft in (0, 6):
                    for nt in range(2):
                        pending.append(
                            lambda b=b + 1, ft=ft, nt=nt: qkT_group(b, ft, nt)
                        )
                for tt in range(TT):
                    pending.append(lambda b=b + 1, tt=tt: v_group(b, tt))
                for ft_pair in range(1, 6):
                    for ft in (ft_pair, 6 + ft_pair):
                        for nt in range(2):
                            pending.append(
                                lambda b=b + 1, ft=ft, nt=nt: qkT_group(b, ft, nt)
                            )
            for h in range(H):
                attn_head(b, h)
                drain(2)
            if b == BL - 1:
                drain(len(pending))
                # pipelined tail: alternate psum pools so copy-out of one
                # proj group overlaps the matmuls of the next
                for tt in range(TT):
                    proj_group(b, tt, pool=(psA if tt % 2 == 0 else psLIN))
            else:
                for tt in range(TT):
                    pending.append(lambda b=b, tt=tt: proj_group(b, tt))

    nc.finalize()
    return nc


_GRAPH = None
LAST_EXEC_TIME_NS = None
LAST_RESULTS = None


def kernel(x, w_qkv, w_proj, b_proj):
    global _GRAPH, LAST_EXEC_TIME_NS, LAST_RESULTS
    import os
    from concourse.bass_utils import run_bass_kernel_spmd

    x = np.asarray(x, dtype=np.float32)
    w_qkv = np.asarray(w_qkv, dtype=np.float32)
    w_proj = np.asarray(w_proj, dtype=np.float32)
    b_proj = np.asarray(b_proj, dtype=np.float32)

    # shard: batches 2i, 2i+1 -> core i; pre-transpose x to [BL, C, N]
    xT = np.ascontiguousarray(
        x.reshape(NCORES, BL, N, C).transpose(0, 1, 3, 2)
    ).astype(BF16)
    wqkvT = np.ascontiguousarray(w_qkv.T).astype(BF16)
    wprojT = np.ascontiguousarray(w_proj.T).astype(BF16)
    bp = np.ascontiguousarray(b_proj.reshape(1, C)).astype(BF16)

    if _GRAPH is None:
        _GRAPH = _build_graph()

    in_maps = [
        {"xT": xT[i], "wqkvT": wqkvT, "wprojT": wprojT, "bproj": bp}
        for i in range(NCORES)
    ]
    trace = os.environ.get("BASS_KERNEL_TRACE") == "1"
    tmpdir = os.environ.get("BASS_KERNEL_TRACE_DIR") if trace else None
    if tmpdir:
        import shutil

        shutil.rmtree(tmpdir, ignore_errors=True)
        os.makedirs(tmpdir, exist_ok=True)
    res = run_bass_kernel_spmd(
        _GRAPH, in_maps, core_ids=list(range(NCORES)), trace=trace, tmpdir=tmpdir
    )
    LAST_EXEC_TIME_NS = res.exec_time_ns
    LAST_RESULTS = res
    out = np.concatenate([res.results[i]["out"] for i in range(NCORES)], axis=0)
    return out.astype(np.float32)


# revision 43
# speedup vs baseline: 1.2349x; 1.2168x over previous
"""Multi-head attention (B=16, N=1024, C=768, H=12) on 8 TRN2 NeuronCores.

Strategy: pure data-parallel over batch (2 batches per core, no collectives).
All matmuls run in bf16 (1 PE cycle/row vs 4 for fp32; rel err ~6e-3).

Per-core pipeline, per batch b (layouts chosen so no transposes are needed):
  1. qkT  [1536, 1024]  = w_qkv[0:1536] @ x[b].T        (feature-major Q,K)
  2. vaug [1024, 12*65] = x[b] @ w_qkv[1536:].T         (token-major V, with
     a ones-column per head -> softmax denominators fall out of the PV matmul)
  3. per head h: S.T = kT.T @ qT (PE), P = exp(S.T * scale) (ACT, no
     max-subtraction needed: logits ~ N(0,1)), PV: outT[65, q] = vaug.T @ P
     accumulated over k tiles.  Row 64 of PV psum = softmax denominator.
     Normalize: reciprocal_approx_fast on the denom row, bf16 cast,
     broadcast across 64 partitions via a K=1 ones matmul, multiply.
  4. proj: out[tok, 768] = attn_outT.T @ w_proj.T + bias (bias folded into
     the matmul as an extra K=1 ones row).

PE/ACT overlap: attention is ACT(exp)-bound, so the projection matmul groups
of the previous batch and the QKV matmul groups of the next batch are
interleaved into the attention head loop via a pending-work queue.
"""

from collections import deque

import numpy as np
import ml_dtypes

B, N, C = 16, 1024, 768
H, HD = 12, 64
NCORES = 8
BL = B // NCORES  # batches per core
SCALE = HD ** -0.5

BF16 = ml_dtypes.bfloat16


def _build_graph():
    import concourse.mybir as mybir
    import concourse.tile as tile
    from concourse import bacc
    from concourse.bass import ds
    from contextlib import ExitStack

    bf = mybir.dt.bfloat16
    f32 = mybir.dt.float32
    Exp = mybir.ActivationFunctionType.Exp

    nc = bacc.Bacc(
        "TRN2", target_bir_lowering=False, debug=False, num_devices=NCORES
    )
    xT_ext = nc.declare_dram_parameter("xT", [BL, C, N], bf, isOutput=False)
    wqkvT_ext = nc.declare_dram_parameter("wqkvT", [C, 3 * C], bf, isOutput=False)
    wprojT_ext = nc.declare_dram_parameter("wprojT", [C, C], bf, isOutput=False)
    bproj_ext = nc.declare_dram_parameter("bproj", [1, C], bf, isOutput=False)
    out_ext = nc.declare_dram_parameter("out", [BL, N, C], f32, isOutput=True)

    CT = C // 128  # 6 input-channel tiles
    TT = N // 128  # 8 token tiles

    with tile.TileContext(nc) as tc, ExitStack() as ctx:
        const = ctx.enter_context(tc.tile_pool(name="const", bufs=1))
        xt_pool = ctx.enter_context(tc.tile_pool(name="xt", bufs=2 * CT))
        qk_pool = ctx.enter_context(tc.tile_pool(name="qk", bufs=24))
        va_pool = ctx.enter_context(tc.tile_pool(name="va", bufs=2 * TT))
        aoT_pool = ctx.enter_context(tc.tile_pool(name="aoT", bufs=12))
        aoU_pool = ctx.enter_context(tc.tile_pool(name="aoU", bufs=4))
        p_pool = ctx.enter_context(tc.tile_pool(name="pp", bufs=3))
        eps_pool = ctx.enter_context(tc.tile_pool(name="eps", bufs=2))
        osb_pool = ctx.enter_context(tc.tile_pool(name="osb", bufs=2))
        # PSUM budget (8 banks): st 4 + pv 2 + lin 1 + bc 1.
        # psA holds the [128,512] ST tiles (deep pipeline so ACT never
        # starves) and doubles as the psum pool for startup/tail linear
        # groups; psLIN (1 buf) serves the linear groups interleaved into
        # the attention loop.
        psA = ctx.enter_context(tc.tile_pool(name="psA", bufs=4, space="PSUM"))
        psPV = ctx.enter_context(tc.tile_pool(name="psPV", bufs=2, space="PSUM"))
        psLIN = ctx.enter_context(tc.tile_pool(name="psLIN", bufs=1, space="PSUM"))
        psBC = ctx.enter_context(tc.tile_pool(name="psBC", bufs=1, space="PSUM"))

        # --- constants (DMAs issued later, in startup-priority order) ---
        wq = [const.tile([128, 3 * C], bf, name=f"wq{i}") for i in range(CT)]
        wp = [const.tile([128, C], bf, name=f"wp{i}") for i in range(CT)]
        bpr = const.tile([1, C], bf, name="bpr")
        ones_tok = const.tile([1, 128], bf, name="ones_tok")
        nc.vector.memset(ones_tok[:], 1.0)
        ones64 = const.tile([1, 64], bf, name="ones64")
        nc.vector.memset(ones64[:], 1.0)

        def load_weights_qkv():
            # chunked by q/k/v feature block so the first qkT matmul groups
            # only wait on the chunk they read
            for blk in range(3):
                for i in range(CT):
                    nc.sync.dma_start(
                        wq[i][:, ds(blk * C, C)],
                        wqkvT_ext[ds(i * 128, 128), ds(blk * C, C)],
                    )

        def load_weights_proj():
            for i in range(CT):
                nc.sync.dma_start(wp[i][:], wprojT_ext[ds(i * 128, 128), :])
            nc.sync.dma_start(bpr[:], bproj_ext[:])

        # per-batch persistent tiles
        xt = {}
        qk = {}
        va = {}
        aoT = {}
        for b in range(BL):
            xt[b] = [
                xt_pool.tile([128, N], bf, tag="xt", name=f"xt{b}_{i}")
                for i in range(CT)
            ]
            qk[b] = [
                qk_pool.tile([128, N], bf, tag="qk", name=f"qk{b}_{f}")
                for f in range(12)
            ]
            va[b] = [
                va_pool.tile([128, H, 65], bf, tag="va", name=f"va{b}_{t}")
                for t in range(TT)
            ]
            aoT[b] = [
                aoT_pool.tile([128, N], bf, tag="aoT", name=f"aoT{b}_{i}")
                for i in range(CT)
            ]

        def load_xt(b):
            for i in range(CT):
                nc.sync.dma_start(xt[b][i][:], xT_ext[b, ds(i * 128, 128), :])

        def qkT_group(b, ft, nt, pool=None):
            pool = pool or psLIN
            ps = pool.tile(
                [128, 512], f32, tag="st" if pool is psA else "lin",
                name=f"psqk{b}_{ft}_{nt}",
            )
            for ci in range(CT):
                nc.tensor.matmul(
                    ps[:],
                    lhsT=wq[ci][:, ds(ft * 128, 128)],
                    rhs=xt[b][ci][:, ds(nt * 512, 512)],
                    start=(ci == 0),
                    stop=(ci == CT - 1),
                )
            nc.vector.tensor_copy(qk[b][ft][:, ds(nt * 512, 512)], ps[:])

        def v_group(b, tt, pool=None):
            pool = pool or psLIN
            tg = "st" if pool is psA else "lin"
            ps0 = pool.tile([128, 512], f32, tag=tg, name=f"psv{b}_{tt}a")
            ps1 = pool.tile([128, 256], f32, tag=tg, name=f"psv{b}_{tt}b")
            for ci in range(CT):
                nc.tensor.matmul(
                    ps0[:],
                    lhsT=xt[b][ci][:, ds(tt * 128, 128)],
                    rhs=wq[ci][:, ds(2 * C, 512)],
                    start=(ci == 0),
                    stop=(ci == CT - 1),
                )
                nc.tensor.matmul(
                    ps1[:],
                    lhsT=xt[b][ci][:, ds(tt * 128, 128)],
                    rhs=wq[ci][:, ds(2 * C + 512, 256)],
                    start=(ci == 0),
                    stop=(ci == CT - 1),
                )
            nc.vector.memset(va[b][tt][:, :, ds(64, 1)], 1.0)
            nc.vector.tensor_copy(
                va[b][tt][:, ds(0, 8), ds(0, 64)],
                ps0[:].rearrange("p (h d) -> p h d", d=64),
            )
            nc.vector.tensor_copy(
                va[b][tt][:, ds(8, 4), ds(0, 64)],
                ps1[:].rearrange("p (h d) -> p h d", d=64),
            )

        def proj_group(b, tt, pool=None):
            pool = pool or psLIN
            tg = "st" if pool is psA else "lin"
            ps0 = pool.tile([128, 512], f32, tag=tg, name=f"pso{b}_{tt}a")
            ps1 = pool.tile([128, 256], f32, tag=tg, name=f"pso{b}_{tt}b")
            for ci in range(CT):
                nc.tensor.matmul(
                    ps0[:],
                    lhsT=aoT[b][ci][:, ds(tt * 128, 128)],
                    rhs=wp[ci][:, ds(0, 512)],
                    start=(ci == 0),
                    stop=False,
                )
                nc.tensor.matmul(
                    ps1[:],
                    lhsT=aoT[b][ci][:, ds(tt * 128, 128)],
                    rhs=wp[ci][:, ds(512, 256)],
                    start=(ci == 0),
                    stop=False,
                )
            nc.tensor.matmul(
                ps0[:], lhsT=ones_tok[:], rhs=bpr[:, ds(0, 512)],
                start=False, stop=True,
            )
            nc.tensor.matmul(
                ps1[:], lhsT=ones_tok[:], rhs=bpr[:, ds(512, 256)],
                start=False, stop=True,
            )
            osb = osb_pool.tile([128, C], f32, tag="osb", name=f"osb{b}_{tt}")
            nc.vector.tensor_copy(osb[:, ds(0, 512)], ps0[:])
            nc.vector.tensor_copy(osb[:, ds(512, 256)], ps1[:])
            nc.sync.dma_start(out_ext[b, ds(tt * 128, 128), :], osb[:])

        pending = deque()

        def drain(k):
            for _ in range(min(k, len(pending))):
                pending.popleft()()

        def attn_kt(b, h, st8, kt, pv):
            # one k-tile step of head h: ST matmuls, exp, PV accumulate
            q_tile = qk[b][h // 2]
            k_tile = qk[b][6 + h // 2]
            row = (h % 2) * 64
            st = [
                psA.tile([128, 512], f32, tag="st", name=f"st{b}_{h}_{kt}_{qc}")
                for qc in range(2)
            ]
            for qc in range(2):
                nc.tensor.matmul(
                    st[qc][:],
                    lhsT=k_tile[ds(row, 64), ds(kt * 128, 128)],
                    rhs=q_tile[ds(row, 64), ds(qc * 512, 512)],
                    start=True,
                    stop=True,
                )
            pt = p_pool.tile([128, N], bf, tag="pt", name=f"pt{b}_{h}_{kt}")
            for qc in range(2):
                nc.scalar.activation(
                    pt[:, ds(qc * 512, 512)], st[qc][:], Exp, scale=SCALE
                )
            for qc in range(2):
                nc.tensor.matmul(
                    pv[qc][:],
                    lhsT=va[b][kt][:, h, :],
                    rhs=pt[:, ds(qc * 512, 512)],
                    start=(kt == 0),
                    stop=(kt == TT - 1),
                )

        def head_start(b, h):
            # allocate this head's state and emit its first k-tile step so
            # ACT has work queued across the previous head's epilogue
            pv = [
                psPV.tile([65, 512], f32, tag="pv", name=f"pv{b}_{h}_{qc}")
                for qc in range(2)
            ]
            attn_kt(b, h, None, 0, pv)
            return pv

        def head_rest(b, h, pv):
            for kt in range(1, TT):
                attn_kt(b, h, None, kt, pv)
                if kt == 3:
                    drain(1)
            aoU = aoU_pool.tile([64, N], bf, tag="aoU", name=f"aoU{b}_{h}")
            den = eps_pool.tile([1, N], f32, tag="den", name=f"den{b}_{h}")
            for qc in range(2):
                nc.vector.tensor_copy(
                    aoU[:, ds(qc * 512, 512)], pv[qc][ds(0, 64), :]
                )
                nc.vector.tensor_copy(
                    den[:, ds(qc * 512, 512)], pv[qc][ds(64, 1), :]
                )
            return aoU, den

        def head_epilogue(b, h, aoU, den):
            nc.vector.reciprocal_approx_fast(den[:], den[:])
            recb = eps_pool.tile([1, N], bf, tag="recb", name=f"recb{b}_{h}")
            nc.vector.tensor_copy(recb[:], den[:])
            row = (h % 2) * 64
            ao_tile = aoT[b][h // 2]
            for hf in range(2):
                bc = psBC.tile([64, 512], f32, tag="bc", name=f"bc{b}_{h}_{hf}")
                nc.tensor.matmul(
                    bc[:], lhsT=ones64[:], rhs=recb[:, ds(hf * 512, 512)],
                    start=True, stop=True,
                )
                nc.vector.tensor_mul(
                    ao_tile[ds(row, 64), ds(hf * 512, 512)],
                    aoU[:, ds(hf * 512, 512)],
                    bc[:],
                )

        # --- schedule ---
        # startup: only what head 0 needs up front (q/k tiles ft0+ft6, all
        # of V); the remaining qkT groups of batch 0 are interleaved into
        # the early attention heads, ordered so head h's tiles are ready
        # ~2 heads ahead of their first use.
        load_xt(0)
        load_weights_qkv()
        for ft in (0, 6):
            for nt in range(2):
                qkT_group(0, ft, nt, pool=psA)
        for tt in range(TT):
            v_group(0, tt, pool=psA)
        load_weights_proj()
        for ft_pair in range(1, 6):
            for ft in (ft_pair, 6 + ft_pair):
                for nt in range(2):
                    pending.append(lambda ft=ft, nt=nt: qkT_group(0, ft, nt))

        for b in range(BL):
            if b + 1 < BL:
                load_xt(b + 1)
                # order for batch b+1's head 0: ft0+ft6 first, then all of
                # V, then the remaining ft pairs in head-use order
                for ft in (0, 6):
                    for nt in range(2):
                        pending.append(
                            lambda b=b + 1, ft=ft, nt=nt: qkT_group(b, ft, nt)
                        )
                for tt in range(TT):
                    pending.append(lambda b=b + 1, tt=tt: v_group(b, tt))
                for ft_pair in range(1, 6):
                    for ft in (ft_pair, 6 + ft_pair):
                        for nt in range(2):
                            pending.append(
                                lambda b=b + 1, ft=ft, nt=nt: qkT_group(b, ft, nt)
                            )
            # software-pipelined head loop: the next head's first k-tile is
            # emitted before the current head's epilogue so ACT never idles
            # across head boundaries
            pv_cur = head_start(b, 0)
            for h in range(H):
                aoU, den = head_rest(b, h, pv_cur)
                if h + 1 < H:
                    pv_cur = head_start(b, h + 1)
                head_epilogue(b, h, aoU, den)
                drain(2)
            if b == BL - 1:
                drain(len(pending))
                # pipelined tail: alternate psum pools so copy-out of one
                # proj group overlaps the matmuls of the next
                for tt in range(TT):
                    proj_group(b, tt, pool=(psA if tt % 2 == 0 else psLIN))
            else:
                for tt in range(TT):
                    pending.append(lambda b=b, tt=tt: proj_group(b, tt))

    nc.finalize()
    return nc


_GRAPH = None
LAST_EXEC_TIME_NS = None
LAST_RESULTS = None


def kernel(x, w_qkv, w_proj, b_proj):
    global _GRAPH, LAST_EXEC_TIME_NS, LAST_RESULTS
    import os
    from concourse.bass_utils import run_bass_kernel_spmd

    x = np.asarray(x, dtype=np.float32)
    w_qkv = np.asarray(w_qkv, dtype=np.float32)
    w_proj = np.asarray(w_proj, dtype=np.float32)
    b_proj = np.asarray(b_proj, dtype=np.float32)

    # shard: batches 2i, 2i+1 -> core i; pre-transpose x to [BL, C, N]
    xT = np.ascontiguousarray(
        x.reshape(NCORES, BL, N, C).transpose(0, 1, 3, 2)
    ).astype(BF16)
    wqkvT = np.ascontiguousarray(w_qkv.T).astype(BF16)
    wprojT = np.ascontiguousarray(w_proj.T).astype(BF16)
    bp = np.ascontiguousarray(b_proj.reshape(1, C)).astype(BF16)

    if _GRAPH is None:
        _GRAPH = _build_graph()

    in_maps = [
        {"xT": xT[i], "wqkvT": wqkvT, "wprojT": wprojT, "bproj": bp}
        for i in range(NCORES)
    ]
    trace = os.environ.get("BASS_KERNEL_TRACE") == "1"
    tmpdir = os.environ.get("BASS_KERNEL_TRACE_DIR") if trace else None
    if tmpdir:
        import shutil

        shutil.rmtree(tmpdir, ignore_errors=True)
        os.makedirs(tmpdir, exist_ok=True)
    res = run_bass_kernel_spmd(
        _GRAPH, in_maps, core_ids=list(range(NCORES)), trace=trace, tmpdir=tmpdir
    )
    LAST_EXEC_TIME_NS = res.exec_time_ns
    LAST_RESULTS = res
    out = np.concatenate([res.results[i]["out"] for i in range(NCORES)], axis=0)
    return out.astype(np.float32)


# revision 44
# speedup vs baseline: 1.2419x; 1.0056x over previous
"""Multi-head attention (B=16, N=1024, C=768, H=12) on 8 TRN2 NeuronCores.

Strategy: pure data-parallel over batch (2 batches per core, no collectives).
All matmuls run in bf16 (1 PE cycle/row vs 4 for fp32; rel err ~6e-3).

Per-core pipeline, per batch b (layouts chosen so no transposes are needed):
  1. qkT  [1536, 1024]  = w_qkv[0:1536] @ x[b].T        (feature-major Q,K)
  2. vaug [1024, 12*65] = x[b] @ w_qkv[1536:].T         (token-major V, with
     a ones-column per head -> softmax denominators fall out of the PV matmul)
  3. per head h: S.T = kT.T @ qT (PE), P = exp(S.T * scale) (ACT, no
     max-subtraction needed: logits ~ N(0,1)), PV: outT[65, q] = vaug.T @ P
     accumulated over k tiles.  Row 64 of PV psum = softmax denominator.
     Normalize: reciprocal_approx_fast on the denom row, bf16 cast,
     broadcast across 64 partitions via a K=1 ones matmul, multiply.
  4. proj: out[tok, 768] = attn_outT.T @ w_proj.T + bias (bias folded into
     the matmul as an extra K=1 ones row).

PE/ACT overlap: attention is ACT(exp)-bound, so the projection matmul groups
of the previous batch and the QKV matmul groups of the next batch are
interleaved into the attention head loop via a pending-work queue.
"""

from collections import deque

import numpy as np
import ml_dtypes

B, N, C = 16, 1024, 768
H, HD = 12, 64
NCORES = 8
BL = B // NCORES  # batches per core
SCALE = HD ** -0.5

BF16 = ml_dtypes.bfloat16


def _build_graph():
    import concourse.mybir as mybir
    import concourse.tile as tile
    from concourse import bacc
    from concourse.bass import ds
    from contextlib import ExitStack

    bf = mybir.dt.bfloat16
    f32 = mybir.dt.float32
    Exp = mybir.ActivationFunctionType.Exp

    nc = bacc.Bacc(
        "TRN2", target_bir_lowering=False, debug=False, num_devices=NCORES
    )
    xT_ext = nc.declare_dram_parameter("xT", [BL, C, N], bf, isOutput=False)
    wqkvT_ext = nc.declare_dram_parameter("wqkvT", [C, 3 * C], bf, isOutput=False)
    wprojT_ext = nc.declare_dram_parameter("wprojT", [C, C], bf, isOutput=False)
    bproj_ext = nc.declare_dram_parameter("bproj", [1, C], bf, isOutput=False)
    out_ext = nc.declare_dram_parameter("out", [BL, N, C], f32, isOutput=True)

    CT = C // 128  # 6 input-channel tiles
    TT = N // 128  # 8 token tiles

    with tile.TileContext(nc) as tc, ExitStack() as ctx:
        const = ctx.enter_context(tc.tile_pool(name="const", bufs=1))
        xt_pool = ctx.enter_context(tc.tile_pool(name="xt", bufs=2 * CT))
        qk_pool = ctx.enter_context(tc.tile_pool(name="qk", bufs=24))
        va_pool = ctx.enter_context(tc.tile_pool(name="va", bufs=2 * TT))
        aoT_pool = ctx.enter_context(tc.tile_pool(name="aoT", bufs=12))
        aoU_pool = ctx.enter_context(tc.tile_pool(name="aoU", bufs=4))
        p_pool = ctx.enter_context(tc.tile_pool(name="pp", bufs=4))
        eps_pool = ctx.enter_context(tc.tile_pool(name="eps", bufs=3))
        osb_pool = ctx.enter_context(tc.tile_pool(name="osb", bufs=2))
        # PSUM budget (8 banks): st 4 + pv 2 + lin 1 + bc 1.
        # psA holds the [128,512] ST tiles (deep pipeline so ACT never
        # starves) and doubles as the psum pool for startup/tail linear
        # groups; psLIN (1 buf) serves the linear groups interleaved into
        # the attention loop.
        psA = ctx.enter_context(tc.tile_pool(name="psA", bufs=4, space="PSUM"))
        psPV = ctx.enter_context(tc.tile_pool(name="psPV", bufs=2, space="PSUM"))
        psLIN = ctx.enter_context(tc.tile_pool(name="psLIN", bufs=1, space="PSUM"))
        psBC = ctx.enter_context(tc.tile_pool(name="psBC", bufs=1, space="PSUM"))

        # --- constants (DMAs issued later, in startup-priority order) ---
        wq = [const.tile([128, 3 * C], bf, name=f"wq{i}") for i in range(CT)]
        wp = [const.tile([128, C], bf, name=f"wp{i}") for i in range(CT)]
        bpr = const.tile([1, C], bf, name="bpr")
        ones_tok = const.tile([1, 128], bf, name="ones_tok")
        nc.vector.memset(ones_tok[:], 1.0)
        ones64 = const.tile([1, 64], bf, name="ones64")
        nc.vector.memset(ones64[:], 1.0)

        def load_weights_qkv():
            # chunked by q/k/v feature block so the first qkT matmul groups
            # only wait on the chunk they read
            for blk in range(3):
                for i in range(CT):
                    nc.sync.dma_start(
                        wq[i][:, ds(blk * C, C)],
                        wqkvT_ext[ds(i * 128, 128), ds(blk * C, C)],
                    )

        def load_weights_proj():
            for i in range(CT):
                nc.sync.dma_start(wp[i][:], wprojT_ext[ds(i * 128, 128), :])
            nc.sync.dma_start(bpr[:], bproj_ext[:])

        # per-batch persistent tiles
        xt = {}
        qk = {}
        va = {}
        aoT = {}
        for b in range(BL):
            xt[b] = [
                xt_pool.tile([128, N], bf, tag="xt", name=f"xt{b}_{i}")
                for i in range(CT)
            ]
            qk[b] = [
                qk_pool.tile([128, N], bf, tag="qk", name=f"qk{b}_{f}")
                for f in range(12)
            ]
            va[b] = [
                va_pool.tile([128, H, 65], bf, tag="va", name=f"va{b}_{t}")
                for t in range(TT)
            ]
            aoT[b] = [
                aoT_pool.tile([128, N], bf, tag="aoT", name=f"aoT{b}_{i}")
                for i in range(CT)
            ]

        def load_xt(b):
            for i in range(CT):
                nc.sync.dma_start(xt[b][i][:], xT_ext[b, ds(i * 128, 128), :])

        def qkT_group(b, ft, nt, pool=None):
            pool = pool or psLIN
            ps = pool.tile(
                [128, 512], f32, tag="st" if pool is psA else "lin",
                name=f"psqk{b}_{ft}_{nt}",
            )
            for ci in range(CT):
                nc.tensor.matmul(
                    ps[:],
                    lhsT=wq[ci][:, ds(ft * 128, 128)],
                    rhs=xt[b][ci][:, ds(nt * 512, 512)],
                    start=(ci == 0),
                    stop=(ci == CT - 1),
                )
            nc.vector.tensor_copy(qk[b][ft][:, ds(nt * 512, 512)], ps[:])

        def v_group(b, tt, pool=None):
            pool = pool or psLIN
            tg = "st" if pool is psA else "lin"
            ps0 = pool.tile([128, 512], f32, tag=tg, name=f"psv{b}_{tt}a")
            ps1 = pool.tile([128, 256], f32, tag=tg, name=f"psv{b}_{tt}b")
            for ci in range(CT):
                nc.tensor.matmul(
                    ps0[:],
                    lhsT=xt[b][ci][:, ds(tt * 128, 128)],
                    rhs=wq[ci][:, ds(2 * C, 512)],
                    start=(ci == 0),
                    stop=(ci == CT - 1),
                )
                nc.tensor.matmul(
                    ps1[:],
                    lhsT=xt[b][ci][:, ds(tt * 128, 128)],
                    rhs=wq[ci][:, ds(2 * C + 512, 256)],
                    start=(ci == 0),
                    stop=(ci == CT - 1),
                )
            nc.vector.memset(va[b][tt][:, :, ds(64, 1)], 1.0)
            nc.vector.tensor_copy(
                va[b][tt][:, ds(0, 8), ds(0, 64)],
                ps0[:].rearrange("p (h d) -> p h d", d=64),
            )
            nc.vector.tensor_copy(
                va[b][tt][:, ds(8, 4), ds(0, 64)],
                ps1[:].rearrange("p (h d) -> p h d", d=64),
            )

        def proj_group(b, tt, pool=None):
            pool = pool or psLIN
            tg = "st" if pool is psA else "lin"
            ps0 = pool.tile([128, 512], f32, tag=tg, name=f"pso{b}_{tt}a")
            ps1 = pool.tile([128, 256], f32, tag=tg, name=f"pso{b}_{tt}b")
            for ci in range(CT):
                nc.tensor.matmul(
                    ps0[:],
                    lhsT=aoT[b][ci][:, ds(tt * 128, 128)],
                    rhs=wp[ci][:, ds(0, 512)],
                    start=(ci == 0),
                    stop=False,
                )
                nc.tensor.matmul(
                    ps1[:],
                    lhsT=aoT[b][ci][:, ds(tt * 128, 128)],
                    rhs=wp[ci][:, ds(512, 256)],
                    start=(ci == 0),
                    stop=False,
                )
            nc.tensor.matmul(
                ps0[:], lhsT=ones_tok[:], rhs=bpr[:, ds(0, 512)],
                start=False, stop=True,
            )
            nc.tensor.matmul(
                ps1[:], lhsT=ones_tok[:], rhs=bpr[:, ds(512, 256)],
                start=False, stop=True,
            )
            osb = osb_pool.tile([128, C], f32, tag="osb", name=f"osb{b}_{tt}")
            nc.vector.tensor_copy(osb[:, ds(0, 512)], ps0[:])
            nc.vector.tensor_copy(osb[:, ds(512, 256)], ps1[:])
            nc.sync.dma_start(out_ext[b, ds(tt * 128, 128), :], osb[:])

        pending = deque()

        def drain(k):
            for _ in range(min(k, len(pending))):
                pending.popleft()()

        def attn_kt(b, h, st8, kt, pv):
            # one k-tile step of head h: ST matmuls, exp, PV accumulate
            q_tile = qk[b][h // 2]
            k_tile = qk[b][6 + h // 2]
            row = (h % 2) * 64
            st = [
                psA.tile([128, 512], f32, tag="st", name=f"st{b}_{h}_{kt}_{qc}")
                for qc in range(2)
            ]
            for qc in range(2):
                nc.tensor.matmul(
                    st[qc][:],
                    lhsT=k_tile[ds(row, 64), ds(kt * 128, 128)],
                    rhs=q_tile[ds(row, 64), ds(qc * 512, 512)],
                    start=True,
                    stop=True,
                )
            pt = p_pool.tile([128, N], bf, tag="pt", name=f"pt{b}_{h}_{kt}")
            for qc in range(2):
                nc.scalar.activation(
                    pt[:, ds(qc * 512, 512)], st[qc][:], Exp, scale=SCALE
                )
            for qc in range(2):
                nc.tensor.matmul(
                    pv[qc][:],
                    lhsT=va[b][kt][:, h, :],
                    rhs=pt[:, ds(qc * 512, 512)],
                    start=(kt == 0),
                    stop=(kt == TT - 1),
                )

        def head_start(b, h):
            # allocate this head's state and emit its first k-tile step so
            # ACT has work queued across the previous head's epilogue
            pv = [
                psPV.tile([65, 512], f32, tag="pv", name=f"pv{b}_{h}_{qc}")
                for qc in range(2)
            ]
            attn_kt(b, h, None, 0, pv)
            return pv

        def head_rest(b, h, pv):
            for kt in range(1, TT):
                attn_kt(b, h, None, kt, pv)
                if kt == 3:
                    drain(1)
            aoU = aoU_pool.tile([64, N], bf, tag="aoU", name=f"aoU{b}_{h}")
            den = eps_pool.tile([1, N], f32, tag="den", name=f"den{b}_{h}")
            for qc in range(2):
                nc.vector.tensor_copy(
                    aoU[:, ds(qc * 512, 512)], pv[qc][ds(0, 64), :]
                )
                nc.vector.tensor_copy(
                    den[:, ds(qc * 512, 512)], pv[qc][ds(64, 1), :]
                )
            return aoU, den

        def head_epilogue(b, h, aoU, den):
            nc.vector.reciprocal_approx_fast(den[:], den[:])
            recb = eps_pool.tile([1, N], bf, tag="recb", name=f"recb{b}_{h}")
            nc.vector.tensor_copy(recb[:], den[:])
            row = (h % 2) * 64
            ao_tile = aoT[b][h // 2]
            for hf in range(2):
                bc = psBC.tile([64, 512], f32, tag="bc", name=f"bc{b}_{h}_{hf}")
                nc.tensor.matmul(
                    bc[:], lhsT=ones64[:], rhs=recb[:, ds(hf * 512, 512)],
                    start=True, stop=True,
                )
                nc.vector.tensor_mul(
                    ao_tile[ds(row, 64), ds(hf * 512, 512)],
                    aoU[:, ds(hf * 512, 512)],
                    bc[:],
                )

        # --- schedule ---
        # startup: only what head 0 needs up front (q/k tiles ft0+ft6, all
        # of V); the remaining qkT groups of batch 0 are interleaved into
        # the early attention heads, ordered so head h's tiles are ready
        # ~2 heads ahead of their first use.
        load_xt(0)
        load_weights_qkv()
        for ft in (0, 6):
            for nt in range(2):
                qkT_group(0, ft, nt, pool=psA)
        for tt in range(TT):
            v_group(0, tt, pool=psA)
        load_weights_proj()
        for ft_pair in range(1, 6):
            for ft in (ft_pair, 6 + ft_pair):
                for nt in range(2):
                    pending.append(lambda ft=ft, nt=nt: qkT_group(0, ft, nt))

        for b in range(BL):
            if b + 1 < BL:
                load_xt(b + 1)
                # order for batch b+1's head 0: ft0+ft6 first, then all of
                # V, then the remaining ft pairs in head-use order
                for ft in (0, 6):
                    for nt in range(2):
                        pending.append(
                            lambda b=b + 1, ft=ft, nt=nt: qkT_group(b, ft, nt)
                        )
                for tt in range(TT):
                    pending.append(lambda b=b + 1, tt=tt: v_group(b, tt))
                for ft_pair in range(1, 6):
                    for ft in (ft_pair, 6 + ft_pair):
                        for nt in range(2):
                            pending.append(
                                lambda b=b + 1, ft=ft, nt=nt: qkT_group(b, ft, nt)
                            )
            # software-pipelined head loop: the next head's first k-tile is
            # emitted before the current head's epilogue so ACT never idles
            # across head boundaries
            pv_cur = head_start(b, 0)
            for h in range(H):
                aoU, den = head_rest(b, h, pv_cur)
                if h + 1 < H:
                    pv_cur = head_start(b, h + 1)
                head_epilogue(b, h, aoU, den)
                drain(2)
            if b == BL - 1:
                drain(len(pending))
                # pipelined tail: alternate psum pools so copy-out of one
                # proj group overlaps the matmuls of the next
                for tt in range(TT):
                    proj_group(b, tt, pool=(psA if tt % 2 == 0 else psLIN))
            else:
                for tt in range(TT):
                    pending.append(lambda b=b, tt=tt: proj_group(b, tt))

    nc.finalize()
    return nc


_GRAPH = None
LAST_EXEC_TIME_NS = None
LAST_RESULTS = None


def kernel(x, w_qkv, w_proj, b_proj):
    global _GRAPH, LAST_EXEC_TIME_NS, LAST_RESULTS
    import os
    from concourse.bass_utils import run_bass_kernel_spmd

    x = np.asarray(x, dtype=np.float32)
    w_qkv = np.asarray(w_qkv, dtype=np.float32)
    w_proj = np.asarray(w_proj, dtype=np.float32)
    b_proj = np.asarray(b_proj, dtype=np.float32)

    # shard: batches 2i, 2i+1 -> core i; pre-transpose x to [BL, C, N]
    xT = np.ascontiguousarray(
        x.reshape(NCORES, BL, N, C).transpose(0, 1, 3, 2)
    ).astype(BF16)
    wqkvT = np.ascontiguousarray(w_qkv.T).astype(BF16)
    wprojT = np.ascontiguousarray(w_proj.T).astype(BF16)
    bp = np.ascontiguousarray(b_proj.reshape(1, C)).astype(BF16)

    if _GRAPH is None:
        _GRAPH = _build_graph()

    in_maps = [
        {"xT": xT[i], "wqkvT": wqkvT, "wprojT": wprojT, "bproj": bp}
        for i in range(NCORES)
    ]
    trace = os.environ.get("BASS_KERNEL_TRACE") == "1"
    tmpdir = os.environ.get("BASS_KERNEL_TRACE_DIR") if trace else None
    if tmpdir:
        import shutil

        shutil.rmtree(tmpdir, ignore_errors=True)
        os.makedirs(tmpdir, exist_ok=True)
    res = run_bass_kernel_spmd(
        _GRAPH, in_maps, core_ids=list(range(NCORES)), trace=trace, tmpdir=tmpdir
    )
    LAST_EXEC_TIME_NS = res.exec_time_ns
    LAST_RESULTS = res
    out = np.concatenate([res.results[i]["out"] for i in range(NCORES)], axis=0)
    return out.astype(np.float32)
